# revision 1
# baseline (speedup 1.0000x reference)
"""Conv2D 3x3 (stride 1, pad 1) Trainium2 Bass kernel.

Problem: x (32, 64, 64, 64) NCHW fp32, weight (128, 64, 3, 3) OIHW, bias (128,).
Output: (32, 128, 64, 64).

Strategy: data-parallel over batch across 8 cores (4 images/core). The host
pre-pads each image channel into a 66x66 zero-ringed layout (+ tail slack) and
pre-rounds x/weights to the PE's fp32r grid (1s+8e+11m, round-to-nearest).
On-chip, partitions 0-63 hold the padded channels and partitions 64-127 hold
the same data shifted down one padded row (a second DMA of the same HBM bytes
at offset 66), so a single K=128 matmul contracts two kernel-row taps at once.
Conv = 6 accumulating fp32r matmuls per 384-pixel PSUM tile (3 paired
ky={0,1} + 3 single ky=2); fp32r runs at full PE rate for moving dim >= 256.
Bias-add fuses into the PSUM->SBUF eviction on the scalar engine.
"""

import numpy as np

import concourse.bass as bass
import concourse.mybir as mybir
import concourse.tile as tile
from concourse import bacc
from concourse.bass_utils import run_bass_kernel_spmd
from concourse.tile_rust import add_dep_helper

N_CORES = 8
NIMG = 4  # images per core
C = 64  # input channels
H = W = 64
O = 128  # output channels
PW = 66  # padded row length
PH = 66  # padded rows
IMG = PH * PW  # 4356 padded elements per channel per image
QTOT = H * PW  # 4224 output positions in padded indexing (64 rows x 66)
# Row-aligned PSUM tiles: 10 groups of 6 output rows + 1 of 4 rows. Row
# alignment lets the eviction compact away the 2 garbage columns per row so
# the output staging buffer (and its store DMA) is fully contiguous.
TILE_ROWS = [6] * 10 + [4]
NQT = len(TILE_ROWS)  # 11
TAIL = 134  # slack so shifted reads stay in-bounds
XCOLS = IMG + TAIL  # 4490
UPLEN = QTOT + 8  # 4232: columns needed in the shifted upper half

F32 = mybir.dt.float32
F32R = mybir.dt.float32r

_CACHED_NC = None


def build_nc():
    nc = bacc.Bacc()
    x_in = nc.declare_dram_parameter("xp", [NIMG, C, XCOLS], F32R, isOutput=False)
    w_in = nc.declare_dram_parameter("wcat", [2 * C, 6, O], F32R, isOutput=False)
    b_in = nc.declare_dram_parameter("bias", [O, 1], F32, isOutput=False)
    out = nc.declare_dram_parameter("out", [NIMG, O, H, W], F32, isOutput=True)

    with tile.TileContext(nc) as tc:
        with (
            tc.tile_pool(name="const", bufs=1) as const_pool,
            tc.tile_pool(name="xp", bufs=4) as x_pool,
            tc.tile_pool(name="osb", bufs=2) as o_pool,
            tc.tile_pool(name="psum0", bufs=4, space="PSUM") as psum0_pool,
            tc.tile_pool(name="psum", bufs=4, space="PSUM") as psum_pool,
        ):
            wcat = const_pool.tile([2 * C, 6, O], F32R)
            bias_t = const_pool.tile([O, 1], F32)
            wcat_dma = nc.sync.dma_start(wcat[:, :, :], w_in[:, :, :])
            nc.sync.dma_start(bias_t[:, :], b_in[:, :])

            # Dummy 1x1 matmul reading only wcat: absorbs the weight-DMA
            # wait so the first real matmul carries a single sync wait (the
            # fused fp32r LDWEIGHTS+MM instruction has one wait slot).
            # (fp32r ISA: innermost free counts must be even, dst partition 0)
            warm = psum_pool.tile([2, 2], F32, tag="acc")
            warm_mm = nc.tensor.matmul(
                warm[:, :], wcat[0:1, 0, 0:2], wcat[0:1, 0, 0:2],
                start=True, stop=True,
            )

            for m in range(NIMG):
                xt = x_pool.tile([128, XCOLS], F32R)
                # lower half: padded image; upper half: same shifted one
                # padded row (pairs kernel rows ky=0/1 in one K=128 matmul).
                # Separate queues (SP HWDGE vs GPSIMD SWDGE) so the two loads
                # run concurrently and never queue behind output stores.
                nc.sync.dma_start(xt[0:C, :], x_in[m, :, :])
                nc.gpsimd.dma_start(
                    xt[C : 2 * C, 0:UPLEN], x_in[m, :, PW : PW + UPLEN]
                )

                osb = o_pool.tile([O, H * W], F32)
                r0 = 0
                for t in range(NQT):
                    rows = TILE_ROWS[t]
                    q0 = r0 * PW
                    qt = rows * PW
                    pool = psum0_pool if t == 0 else psum_pool
                    acc = pool.tile([O, 6 * PW], F32, tag="acc")
                    # ky=2 singles first: they read only the lower xt half,
                    # keeping per-matmul semaphore waits within the fused
                    # fp32r LDWEIGHTS+MM wait-slot budget.
                    for kx in range(3):
                        mm = nc.tensor.matmul(
                            acc[:, 0:qt],
                            wcat[0:C, 3 + kx, :],
                            xt[0:C, q0 + 2 * PW + kx : q0 + 2 * PW + kx + qt],
                            start=(kx == 0),
                            stop=False,
                        )
                        if m == 0 and t == 0 and kx == 0:
                            add_dep_helper(
                                mm.ins, warm_mm.ins, sync=False, reason="warm first"
                            )
                    for kx in range(3):
                        nc.tensor.matmul(
                            acc[:, 0:qt],
                            wcat[:, kx, :],
                            xt[0 : 2 * C, q0 + kx : q0 + kx + qt],
                            start=False,
                            stop=(kx == 2),
                        )
                    # evict + bias add on the scalar engine, dropping the 2
                    # garbage columns per row so osb is contiguous valid data
                    av = acc[:, 0:qt].rearrange("p (r c) -> p r c", c=PW)
                    ov = osb[:, r0 * W : (r0 + rows) * W].rearrange(
                        "p (r c) -> p r c", c=W
                    )
                    nc.scalar.activation(
                        ov[:, :, :],
                        av[:, :, 0:W],
                        mybir.ActivationFunctionType.Identity,
                        bias=bias_t[:, :],
                    )
                    r0 += rows

                # contiguous store on the ACT HWDGE queue
                nc.scalar.dma_start(out[m, :, :, :], osb[:, :])

    nc.compile()
    return nc


def _round_fp32r(a: np.ndarray) -> np.ndarray:
    """Round fp32 to the fp32r grid (11 mantissa bits, RNE)."""
    a = np.ascontiguousarray(a, dtype=np.float32)
    u = a.view(np.uint32)
    low = u & np.uint32(0xFFF)
    lsb = (u >> np.uint32(12)) & np.uint32(1)
    round_up = (low > 0x800) | ((low == 0x800) & (lsb == 1))
    r = (u & np.uint32(0xFFFFF000)) + (round_up.astype(np.uint32) << np.uint32(12))
    return r.view(np.float32)


def _prep_inputs(x, weight, bias):
    x = _round_fp32r(np.asarray(x, dtype=np.float32))
    n = x.shape[0]
    z = np.zeros((n, C, PH, PW), dtype=np.float32)
    z[:, :, 1 : 1 + H, 1 : 1 + W] = x
    xp = np.zeros((n, C, XCOLS), dtype=np.float32)
    xp[:, :, :IMG] = z.reshape(n, C, IMG)

    w_t = _round_fp32r(np.asarray(weight, dtype=np.float32)).transpose(1, 2, 3, 0)
    wcat = np.zeros((2 * C, 6, O), dtype=np.float32)
    wcat[0:C, 0:3, :] = w_t[:, 0, :, :]  # ky=0 (lower half of pairs)
    wcat[C : 2 * C, 0:3, :] = w_t[:, 1, :, :]  # ky=1 (upper half of pairs)
    wcat[0:C, 3:6, :] = w_t[:, 2, :, :]  # ky=2 singles
    b = np.ascontiguousarray(np.asarray(bias, dtype=np.float32).reshape(O, 1))
    return xp, wcat, b


def _in_maps(x, weight, bias):
    xp, wcat, b = _prep_inputs(x, weight, bias)
    return [
        {"xp": xp[i * NIMG : (i + 1) * NIMG], "wcat": wcat, "bias": b}
        for i in range(N_CORES)
    ]


def kernel(x: np.ndarray, weight: np.ndarray, bias: np.ndarray) -> np.ndarray:
    global _CACHED_NC
    if _CACHED_NC is None:
        _CACHED_NC = build_nc()
    res = run_bass_kernel_spmd(_CACHED_NC, _in_maps(x, weight, bias), list(range(N_CORES)))
    return np.concatenate([r["out"] for r in res.results], axis=0)


def run_profiled(x, weight, bias, tmpdir=None):
    """Dev helper: run with NTFF tracing, return BassKernelResults."""
    global _CACHED_NC
    if _CACHED_NC is None:
        _CACHED_NC = build_nc()
    return run_bass_kernel_spmd(
        _CACHED_NC, _in_maps(x, weight, bias), list(range(N_CORES)),
        trace=True, tmpdir=tmpdir,
    )



# revision 2
# speedup vs baseline: 1.3997x; 1.3997x over previous
"""Conv2D 3x3 (stride 1, pad 1) Trainium2 Bass kernel.

Problem: x (32, 64, 64, 64) NCHW fp32, weight (128, 64, 3, 3) OIHW, bias (128,).
Output: (32, 128, 64, 64).

Strategy: data-parallel over batch across 8 cores (4 images/core). The host
pre-pads each image channel into a 66x66 zero-ringed layout (+ tail slack) and
converts x/weights to fp16 (10-bit mantissa; rel-err ~3e-4 vs the 2e-2 budget).
fp16 moving operands stream at 1 cycle/row on the PE (vs 2 for fp32r whose
4-byte elements saturate the SBUF stream port), and fp16 weight loads get the
hardware fast-weight-load path, so per-matmul LDWEIGHTS hides behind the
previous matmul in the PE's reorder window.

On-chip, partitions 0-63 hold the padded channels and partitions 64-127 hold
the same data shifted down one padded row (a second DMA of the same HBM bytes
at offset 66), so a single K=128 matmul contracts two kernel-row taps at once.
Conv = 6 accumulating matmuls per 396-pixel PSUM tile (3 single ky=2 taps with
K=64 + 3 paired ky={0,1} taps with K=128). Bias-add fuses into the PSUM->SBUF
eviction, alternating between the scalar and vector engines; outputs store as
fp16 and are widened to fp32 on the host.
"""

import numpy as np

import concourse.bass as bass
import concourse.mybir as mybir
import concourse.tile as tile
from concourse import bacc
from concourse.bass_utils import run_bass_kernel_spmd

N_CORES = 8
NIMG = 4  # images per core
C = 64  # input channels
H = W = 64
O = 128  # output channels
PW = 66  # padded row length
PH = 66  # padded rows
IMG = PH * PW  # 4356 padded elements per channel per image
QTOT = H * PW  # 4224 output positions in padded indexing (64 rows x 66)
# Row-aligned PSUM tiles: 10 groups of 6 output rows + 1 of 4 rows. Row
# alignment lets the eviction compact away the 2 garbage columns per row so
# the output staging buffer (and its store DMA) is fully contiguous.
TILE_ROWS = [6] * 10 + [4]
NQT = len(TILE_ROWS)  # 11
XCOLS = 4364  # IMG + 8 slack: lower-half matmul reads reach 4358
UPLEN = QTOT + 8  # 4232: columns needed in the shifted upper half
CUT = 34 * PW  # 2244: image-0 first-chunk split so tile 0 starts early
STORE_SPLIT = 36 * W  # store rows 0-35 (tiles 0-5) while 6-10 compute

F16 = mybir.dt.float16
F32 = mybir.dt.float32

_CACHED_NC = None


def build_nc():
    nc = bacc.Bacc()
    x_in = nc.declare_dram_parameter("xp", [NIMG, C, XCOLS], F16, isOutput=False)
    w_in = nc.declare_dram_parameter("wcat", [2 * C, 6, O], F16, isOutput=False)
    b_in = nc.declare_dram_parameter("bias", [O, 1], F32, isOutput=False)
    out = nc.declare_dram_parameter("out", [NIMG, O, H, W], F16, isOutput=True)

    with tile.TileContext(nc) as tc:
        with (
            tc.tile_pool(name="const", bufs=1) as const_pool,
            tc.tile_pool(name="xp", bufs=4) as x_pool,
            tc.tile_pool(name="osb", bufs=2) as o_pool,
            tc.tile_pool(name="psum", bufs=8, space="PSUM") as psum_pool,
        ):
            wcat = const_pool.tile([2 * C, 6, O], F16)
            bias_t = const_pool.tile([O, 1], F32)
            nc.scalar.dma_start(wcat[:, :, :], w_in[:, :, :])
            nc.scalar.dma_start(bias_t[:, :], b_in[:, :])

            for m in range(NIMG):
                xt = x_pool.tile([128, XCOLS], F16)
                # lower half: padded image; upper half: same shifted one
                # padded row (pairs kernel rows ky=0/1 in one K=128 matmul).
                # Separate queues (SP HWDGE vs GPSIMD SWDGE) so the two loads
                # run concurrently and never queue behind output stores.
                if m == 0:
                    # split image 0 so tile 0's matmuls only wait on the
                    # first ~half of the image (tiles 0-4 read cols < CUT)
                    nc.sync.dma_start(xt[0:C, 0:CUT], x_in[m, :, 0:CUT])
                    nc.gpsimd.dma_start(
                        xt[C : 2 * C, 0:CUT], x_in[m, :, PW : PW + CUT]
                    )
                    nc.sync.dma_start(xt[0:C, CUT:XCOLS], x_in[m, :, CUT:XCOLS])
                    nc.gpsimd.dma_start(
                        xt[C : 2 * C, CUT:UPLEN],
                        x_in[m, :, PW + CUT : PW + UPLEN],
                    )
                else:
                    nc.sync.dma_start(xt[0:C, :], x_in[m, :, :])
                    nc.gpsimd.dma_start(
                        xt[C : 2 * C, 0:UPLEN], x_in[m, :, PW : PW + UPLEN]
                    )

                osb = o_pool.tile([O, H * W], F16)
                r0 = 0
                for t in range(NQT):
                    rows = TILE_ROWS[t]
                    q0 = r0 * PW
                    qt = rows * PW
                    acc = psum_pool.tile([O, 6 * PW], F32, tag="acc")
                    # ky=2 singles first: they read only the lower xt half,
                    # so image-0 startup doesn't wait on the upper-half DMA.
                    for kx in range(3):
                        nc.tensor.matmul(
                            acc[:, 0:qt],
                            wcat[0:C, 3 + kx, :],
                            xt[0:C, q0 + 2 * PW + kx : q0 + 2 * PW + kx + qt],
                            start=(kx == 0),
                            stop=False,
                        )
                    for kx in range(3):
                        nc.tensor.matmul(
                            acc[:, 0:qt],
                            wcat[:, kx, :],
                            xt[0 : 2 * C, q0 + kx : q0 + kx + qt],
                            start=False,
                            stop=(kx == 2),
                        )
                    # evict + bias add, dropping the 2 garbage columns per
                    # row so osb is contiguous valid data. Alternate scalar/
                    # vector engines so eviction never gates PSUM recycling.
                    av = acc[:, 0:qt].rearrange("p (r c) -> p r c", c=PW)
                    ov = osb[:, r0 * W : (r0 + rows) * W].rearrange(
                        "p (r c) -> p r c", c=W
                    )
                    if t % 2 == 0:
                        nc.scalar.activation(
                            ov[:, :, :],
                            av[:, :, 0:W],
                            mybir.ActivationFunctionType.Identity,
                            bias=bias_t[:, :],
                        )
                    else:
                        nc.vector.tensor_scalar_add(
                            ov[:, :, :], av[:, :, 0:W], bias_t[:, 0:1]
                        )
                    r0 += rows
                    if r0 * W == STORE_SPLIT:
                        nc.scalar.dma_start(
                            out[m, :, 0 : STORE_SPLIT // W, :],
                            osb[:, 0:STORE_SPLIT].rearrange(
                                "p (r c) -> p r c", c=W
                            ),
                        )

                nc.scalar.dma_start(
                    out[m, :, STORE_SPLIT // W : H, :],
                    osb[:, STORE_SPLIT : H * W].rearrange("p (r c) -> p r c", c=W),
                )

    nc.compile()
    return nc


def _prep_inputs(x, weight, bias):
    x = np.asarray(x, dtype=np.float32)
    n = x.shape[0]
    z = np.zeros((n, C, PH, PW), dtype=np.float16)
    z[:, :, 1 : 1 + H, 1 : 1 + W] = x
    xp = np.zeros((n, C, XCOLS), dtype=np.float16)
    xp[:, :, :IMG] = z.reshape(n, C, IMG)

    w_t = np.asarray(weight, dtype=np.float32).astype(np.float16)
    w_t = w_t.transpose(1, 2, 3, 0)  # [C, ky, kx, O]
    wcat = np.zeros((2 * C, 6, O), dtype=np.float16)
    wcat[0:C, 0:3, :] = w_t[:, 0, :, :]  # ky=0 (lower half of pairs)
    wcat[C : 2 * C, 0:3, :] = w_t[:, 1, :, :]  # ky=1 (upper half of pairs)
    wcat[0:C, 3:6, :] = w_t[:, 2, :, :]  # ky=2 singles
    b = np.ascontiguousarray(np.asarray(bias, dtype=np.float32).reshape(O, 1))
    return xp, wcat, b


def _in_maps(x, weight, bias):
    xp, wcat, b = _prep_inputs(x, weight, bias)
    return [
        {"xp": xp[i * NIMG : (i + 1) * NIMG], "wcat": wcat, "bias": b}
        for i in range(N_CORES)
    ]


def kernel(x: np.ndarray, weight: np.ndarray, bias: np.ndarray) -> np.ndarray:
    global _CACHED_NC
    if _CACHED_NC is None:
        _CACHED_NC = build_nc()
    res = run_bass_kernel_spmd(_CACHED_NC, _in_maps(x, weight, bias), list(range(N_CORES)))
    return np.concatenate(
        [r["out"].astype(np.float32) for r in res.results], axis=0
    )


def run_profiled(x, weight, bias, tmpdir=None):
    """Dev helper: run with NTFF tracing, return BassKernelResults."""
    global _CACHED_NC
    if _CACHED_NC is None:
        _CACHED_NC = build_nc()
    return run_bass_kernel_spmd(
        _CACHED_NC, _in_maps(x, weight, bias), list(range(N_CORES)),
        trace=True, tmpdir=tmpdir,
    )
